# revision 35
# baseline (speedup 1.0000x reference)
"""Trainium2 Bass kernel for CrossDMHAttention (B=131072, single-query cross-attention
with T=24 kv tokens, H=4 heads, head_dim=8, + LN + residual GELU MLP).

Strategy: pure data-parallel over 8 NeuronCores (batch split). Host pre-packs kv
into feature-major chunk layout (f16) so k/v projections need no PE transposes
and stream at 1 cycle/row; q is host-transposed (f16) and preloaded whole.
The projected k/v PSUM tile is immediately copied to SBUF as bf16 (split
ACT/DVE) so every subsequent elementwise op runs from SBUF in 16-bit, where the
DVE gets its 2-4x fast modes and GPSIMD can help (GPSIMD cannot touch PSUM).
Two phases (attention+Wo+LN-stats, then LN-apply+MLP) keep the scalar engine on
one activation table per phase. The MLP tail is batched 4 tiles per group with
2-tile PE transposes whose [64,128] results are copied as two base-0 SBUF
tiles (matmuls from base partition 32/64 operands fault on HW). Output
accumulates in SBUF, ships as one DMA, host unshuffles.
"""

import math

import numpy as np

B, DQ, DKV, T, A, H, O = 131072, 13, 32, 24, 32, 4, 32
HD = A // H
LN_EPS = 1e-5
NCORES = 8
BP = B // NCORES  # rows per core
P = 128
NT = BP // P      # tiles per core
NC = 6            # kv feature chunks (24 tokens * 32 dkv / 128)
G4 = 4            # tiles per batch-group
USE_GPSIMD = True  # offload some SBUF-only elementwise ops to the Pool engine
CSPLIT = 1        # kv chunks copied PSUM->SBUF by DVE (rest by ACT)

_CACHE = {}


def _ap(base, dims, extra_offset=0):
    """Build an AP over base (an AP) with given free [step, count] dims."""
    import concourse.bass as bass
    return bass.AP(tensor=base.tensor, offset=base.offset + extra_offset,
                   ap=[base.ap[0]] + [list(d) for d in dims])


def _build(nt=NT, use_gpsimd=None):
    import concourse.bacc as bacc
    import concourse.tile as tile
    from concourse import mybir

    if use_gpsimd is None:
        use_gpsimd = USE_GPSIMD

    f32 = mybir.dt.float32
    f16 = mybir.dt.float16
    AF = mybir.ActivationFunctionType
    OP = mybir.AluOpType
    AX = mybir.AxisListType

    nc = bacc.Bacc()
    ew = nc.gpsimd if use_gpsimd else nc.vector

    qt_d = nc.dram_tensor("qt", [16, BP], f16, kind="ExternalInput")
    kv_d = nc.dram_tensor("kv_in", [nt, P, NC * P], f16, kind="ExternalInput")
    ident_d = nc.dram_tensor("ident", [128, 128], f16, kind="ExternalInput")
    wq_d = nc.dram_tensor("wq", [16, 32], f16, kind="ExternalInput")
    wkv_d = nc.dram_tensor("wkv", [128, 256], f16, kind="ExternalInput")
    wo_d = nc.dram_tensor("wo", [128, 32], f16, kind="ExternalInput")
    wd1_d = nc.dram_tensor("wd1", [128, 32], f16, kind="ExternalInput")
    wd2_d = nc.dram_tensor("wd2", [128, 32], f16, kind="ExternalInput")
    lnw_d = nc.dram_tensor("lnw", [128, 32], f32, kind="ExternalInput")
    lnb_d = nc.dram_tensor("lnb", [128, 32], f32, kind="ExternalInput")
    out_d = nc.dram_tensor("out", [P, nt * O], f32, kind="ExternalOutput")

    ng = nt // G4

    with tile.TileContext(nc) as tc:
        with (
            tc.tile_pool(name="consts", bufs=1) as consts,
            tc.tile_pool(name="kvload", bufs=5) as kvload,
            tc.tile_pool(name="kvsb", bufs=4) as kvsb,
            tc.tile_pool(name="bigs", bufs=4) as bigs,
            tc.tile_pool(name="trsb", bufs=4) as trsb,
            tc.tile_pool(name="smalls", bufs=6) as smalls,
            tc.tile_pool(name="kvpp", bufs=2, space="PSUM") as kvpp,
            tc.tile_pool(name="sp", bufs=2, space="PSUM") as sp,
        ):
            ident_sb = consts.tile([128, 128], f16)
            wq_sb = consts.tile([16, 32], f16)
            wkv_sb = consts.tile([128, 256], f16)
            wo_sb = consts.tile([128, 32], f16)
            wd1_sb = consts.tile([128, 32], f16)
            wd2_sb = consts.tile([128, 32], f16)
            lnw_sb = consts.tile([128, 32], f32)
            lnb_sb = consts.tile([128, 32], f32)
            qt_all = consts.tile([16, BP], f16)

            qproj_all = consts.tile([128, nt, 32], f16)
            out1_all = consts.tile([128, nt, 32], f32)
            mean_all = consts.tile([128, nt], f32)
            var_all = consts.tile([128, nt], f32)
            std_all = consts.tile([128, nt], f32)
            rstd_all = consts.tile([128, nt], f32)
            out_all = consts.tile([128, nt, 32], f32)
            eps_sb = consts.tile([128, 1], f32)
            nc.vector.memset(eps_sb, LN_EPS)

            nc.sync.dma_start(out=ident_sb, in_=ident_d[:, :])
            nc.sync.dma_start(out=wq_sb, in_=wq_d[:, :])
            nc.sync.dma_start(out=wkv_sb, in_=wkv_d[:, :])
            nc.sync.dma_start(out=wo_sb, in_=wo_d[:, :])
            nc.sync.dma_start(out=wd1_sb, in_=wd1_d[:, :])
            nc.sync.dma_start(out=wd2_sb, in_=wd2_d[:, :])
            nc.sync.dma_start(out=lnw_sb, in_=lnw_d[:, :])
            nc.sync.dma_start(out=lnb_sb, in_=lnb_d[:, :])
            nc.sync.dma_start(out=qt_all, in_=qt_d[:, :])

            # ---- phase A0: q projections, batched bf16 copies ----
            for g in range(ng):
                q4_ps = sp.tile([128, G4, 32], f32, tag="sp")
                for jj in range(G4):
                    i = g * G4 + jj
                    nc.tensor.matmul(q4_ps[:, jj, :],
                                     lhsT=qt_all[:, i * P:(i + 1) * P],
                                     rhs=wq_sb)
                nc.scalar.copy(qproj_all[:, g * G4:(g + 1) * G4, :], q4_ps)

            # ---- phase A: attention + Wo + LN stats ----
            for g in range(ng):
                ctx4 = trsb.tile([128, G4, 32], f16, tag="ctx4")
                for jj in range(G4):
                    i = g * G4 + jj
                    kv_t = kvload.tile([P, NC * P], f16, tag="kv")
                    nc.sync.dma_start(out=kv_t, in_=kv_d[i])

                    kvp = kvpp.tile([P, NC, 4, 2, 32], f32, tag="kvp")
                    for c in range(NC):
                        nc.tensor.matmul(kvp[:, c, :, :, :],
                                         lhsT=kv_t[:, c * P:(c + 1) * P],
                                         rhs=wkv_sb)

                    # ACT copies k,v out of PSUM into fp16 SBUF so the DVE
                    # muls run in 2x packed mode. v is reordered to [H,HD,T]
                    # so the ctx reduce is contiguous-innermost.
                    k_ap = _ap(kvp, [[8, H], [64, T], [1, HD]])
                    v_ap = _ap(kvp, [[8, H], [1, HD], [64, T]],
                               extra_offset=32)
                    ksb = kvsb.tile([P, H, T, HD], f16, tag="ksb")
                    nc.scalar.copy(ksb, k_ap)
                    vsb = kvsb.tile([P, H, HD, T], f16, tag="vsb")
                    nc.scalar.copy(vsb, v_ap)

                    # scores = sum_d q*k, layout [h, t]
                    prod = bigs.tile([P, H, T, HD], f16, tag="prod")
                    qb_ap = _ap(qproj_all, [[8, H], [0, T], [1, HD]],
                                extra_offset=i * 32)
                    nc.vector.tensor_mul(prod, ksb, qb_ap)
                    scores = smalls.tile([P, H, T], f16, tag="scores")
                    with nc.allow_low_precision(reason="8-term f16 sums"):
                        nc.vector.reduce_sum(scores, prod, axis=AX.X)

                    # softmax over t (no max-sub; |scores| <~ 5)
                    exps = smalls.tile([P, H, T], f16, tag="exps")
                    nc.scalar.activation(exps, scores, AF.Exp)
                    denom = smalls.tile([P, H], f32, tag="denom")
                    nc.vector.reduce_sum(denom, exps, axis=AX.X)

                    # ctx = sum_t attn*v; prod2 [H, HD, T] (t contiguous),
                    # split DVE (h 0-1) / GPSIMD (h 2-3)
                    prod2 = bigs.tile([P, H, HD, T], f16, tag="prod2")
                    e_lo = _ap(exps, [[T, 2], [0, HD], [1, T]])
                    e_hi = _ap(exps, [[T, 2], [0, HD], [1, T]],
                               extra_offset=2 * T)
                    nc.vector.tensor_mul(prod2[:, 0:2], vsb[:, 0:2], e_lo)
                    nc.gpsimd.tensor_mul(prod2[:, 2:4], vsb[:, 2:4], e_hi)
                    ctxu = smalls.tile([P, A], f32, tag="ctxu")
                    nc.vector.reduce_sum(ctxu, prod2, axis=AX.X)
                    # ctx = ctxu * (1/denom)  (broadcast over head_dim)
                    rden = smalls.tile([P, H], f32, tag="rden")
                    nc.vector.reciprocal(rden, denom)
                    ew.tensor_tensor(
                        out=ctx4[:, jj, :], in0=ctxu,
                        in1=_ap(rden, [[1, H], [0, HD]]), op=OP.mult)

                # 2-tile transposes; copies split the [64,128] PSUM result
                # into two base-0 SBUF tiles (offset matmuls fault on HW)
                out1_ps = sp.tile([128, G4, O], f32, tag="sp")
                for h2 in range(2):
                    ctxT_ps = sp.tile([64, 128], f16, tag="sp")
                    nc.tensor.transpose(ctxT_ps, ctx4[:, 2 * h2:2 * h2 + 2, :],
                                        ident_sb)
                    for sj in range(2):
                        jj = 2 * h2 + sj
                        ctxT_sb = trsb.tile([32, 128], f16, tag="ctxT")
                        nc.scalar.copy(ctxT_sb, ctxT_ps[sj * 32:sj * 32 + 32])
                        nc.tensor.matmul(out1_ps[:, jj, :], lhsT=ctxT_sb,
                                         rhs=wo_sb[0:32, :])
                o4 = out1_all[:, g * G4:(g + 1) * G4, :]
                nc.vector.tensor_copy(o4, out1_ps)

                # batched LN stats: mean, var = E[x^2] - mean^2
                sums = smalls.tile([P, G4], f32, tag="sums")
                nc.vector.reduce_sum(sums, o4, axis=AX.X)
                sq4 = trsb.tile([128, G4, O], f16, tag="sq4")
                ew.tensor_mul(sq4, o4, o4)
                sqs = smalls.tile([P, G4], f32, tag="sqs")
                nc.vector.reduce_sum(sqs, sq4, axis=AX.X)
                mean4 = mean_all[:, g * G4:(g + 1) * G4]
                ew.tensor_scalar(mean4, sums, 1.0 / O, None,
                                        op0=OP.mult)
                msq = smalls.tile([P, G4], f32, tag="msq")
                ew.tensor_mul(msq, mean4, mean4)
                ew.tensor_scalar(var_all[:, g * G4:(g + 1) * G4], sqs,
                                        1.0 / O, None, op0=OP.mult)
                ew.tensor_sub(var_all[:, g * G4:(g + 1) * G4],
                                     var_all[:, g * G4:(g + 1) * G4], msq)

            # ---- batched rstd (one Sqrt table load) ----
            nc.scalar.activation(std_all, var_all, AF.Sqrt, bias=eps_sb)
            nc.vector.reciprocal(rstd_all, std_all)

            # ---- phase B: LN apply + residual GELU MLP, batched by 4 ----
            for g in range(ng):
                sl = slice(g * G4, (g + 1) * G4)
                o4 = out1_all[:, sl, :]
                mean_b = _ap(mean_all, [[1, G4], [0, O]], extra_offset=g * G4)
                rstd_b = _ap(rstd_all, [[1, G4], [0, O]], extra_offset=g * G4)
                lnw_b = _ap(lnw_sb, [[0, G4], [1, O]])
                lnb_b = _ap(lnb_sb, [[0, G4], [1, O]])

                xc4 = smalls.tile([P, G4, O], f32, tag="xc4")
                nc.vector.tensor_tensor(out=xc4, in0=o4, in1=mean_b,
                                        op=OP.subtract)
                xr4 = smalls.tile([P, G4, O], f32, tag="xr4")
                nc.vector.tensor_tensor(out=xr4, in0=xc4, in1=rstd_b,
                                        op=OP.mult)
                xw4 = smalls.tile([P, G4, O], f32, tag="xw4")
                ew.tensor_tensor(out=xw4, in0=xr4, in1=lnw_b,
                                        op=OP.mult)
                ln4 = trsb.tile([128, G4, O], f16, tag="ln4")
                ew.tensor_tensor(out=ln4, in0=xw4, in1=lnb_b,
                                        op=OP.add)

                h1_ps = sp.tile([128, G4, O], f32, tag="sp")
                for h2 in range(2):
                    lnT_ps = sp.tile([64, 128], f16, tag="sp")
                    nc.tensor.transpose(lnT_ps, ln4[:, 2 * h2:2 * h2 + 2, :],
                                        ident_sb)
                    for sj in range(2):
                        jj = 2 * h2 + sj
                        lnT_sb = trsb.tile([32, 128], f16, tag="lnT")
                        nc.scalar.copy(lnT_sb, lnT_ps[sj * 32:sj * 32 + 32])
                        nc.tensor.matmul(h1_ps[:, jj, :], lhsT=lnT_sb,
                                         rhs=wd1_sb[0:32, :])
                h1_4 = trsb.tile([128, G4, O], f16, tag="h1_4")
                nc.scalar.activation(h1_4, h1_ps, AF.Gelu)

                h2_ps = sp.tile([128, G4, O], f32, tag="sp")
                for h2 in range(2):
                    h1T_ps = sp.tile([64, 128], f16, tag="sp")
                    nc.tensor.transpose(h1T_ps, h1_4[:, 2 * h2:2 * h2 + 2, :],
                                        ident_sb)
                    for sj in range(2):
                        jj = 2 * h2 + sj
                        h1T_sb = trsb.tile([32, 128], f16, tag="h1T")
                        nc.scalar.copy(h1T_sb, h1T_ps[sj * 32:sj * 32 + 32])
                        nc.tensor.matmul(h2_ps[:, jj, :], lhsT=h1T_sb,
                                         rhs=wd2_sb[0:32, :])
                h2_4 = smalls.tile([128, G4, O], f16, tag="h2_4")
                nc.scalar.activation(h2_4, h2_ps, AF.Gelu)

                ew.tensor_add(out_all[:, sl, :], ln4, h2_4)

            nc.sync.dma_start(out=out_d[:, :], in_=out_all)

    nc.compile()
    return nc


def _prep_weights(Wq, Wk, Wv, Wo, ln_w, ln_b, Wd1, Wd2):
    bf = np.float16
    s = 1.0 / math.sqrt(HD)
    wq = np.zeros((16, 32), np.float32)
    wq[:DQ] = np.asarray(Wq, np.float32) * s
    wkv = np.zeros((128, 256), np.float32)
    Wk = np.asarray(Wk, np.float32)
    Wv = np.asarray(Wv, np.float32)
    for tl in range(4):
        wkv[tl * 32:(tl + 1) * 32, tl * 64:tl * 64 + 32] = Wk
        wkv[tl * 32:(tl + 1) * 32, tl * 64 + 32:tl * 64 + 64] = Wv
    lnw = np.broadcast_to(np.asarray(ln_w, np.float32), (128, 32)).copy()
    lnb = np.broadcast_to(np.asarray(ln_b, np.float32), (128, 32)).copy()
    return {
        "ident": np.eye(128, dtype=bf),
        "wq": wq.astype(bf),
        "wkv": wkv.astype(bf),
        "wo": np.tile(np.asarray(Wo, np.float32), (4, 1)).astype(bf),
        "wd1": np.tile(np.asarray(Wd1, np.float32), (4, 1)).astype(bf),
        "wd2": np.tile(np.asarray(Wd2, np.float32), (4, 1)).astype(bf),
        "lnw": lnw,
        "lnb": lnb,
    }


def kernel(query, kv, Wq, Wk, Wv, Wo, ln_w, ln_b, Wd1, Wd2):
    from concourse.bass_utils import run_bass_kernel_spmd

    bf = np.float16

    if "nc" not in _CACHE:
        _CACHE["nc"] = _build()
    nc = _CACHE["nc"]

    query = np.asarray(query, np.float32)
    kv = np.asarray(kv, np.float32)
    w = _prep_weights(Wq, Wk, Wv, Wo, ln_w, ln_b, Wd1, Wd2)

    # q: [B, DQ] -> per-core [16, BP] transposed + padded, bf16
    qt = np.zeros((NCORES, 16, BP), bf)
    qt[:, :DQ, :] = query.reshape(NCORES, BP, DQ).transpose(0, 2, 1).astype(bf)
    # kv: [B, T, DKV] -> per-core [NT, 128(feat r*32+d), 6(c), 128(b)], bf16
    kvp = np.ascontiguousarray(
        kv.reshape(NCORES, NT, P, NC, 4, DKV).transpose(0, 1, 4, 5, 3, 2)
        .astype(bf)
    ).reshape(NCORES, NT, P, NC * P)

    in_maps = []
    for c in range(NCORES):
        m = dict(w)
        m["qt"] = qt[c]
        m["kv_in"] = kvp[c]
        in_maps.append(m)

    res = run_bass_kernel_spmd(nc, in_maps, core_ids=list(range(NCORES)),
                               trace=False)
    _CACHE["last_results"] = res
    out = np.concatenate(
        [r["out"].reshape(P, NT, O).transpose(1, 0, 2).reshape(BP, O)
         for r in res.results], axis=0)
    return out


# revision 36
# speedup vs baseline: 1.1270x; 1.1270x over previous
"""Trainium2 Bass kernel for CrossDMHAttention (B=131072, single-query cross-attention
with T=24 kv tokens, H=4 heads, head_dim=8, + LN + residual GELU MLP).

Strategy: pure data-parallel over 8 NeuronCores (batch split). Host pre-packs kv
into feature-major chunk layout (f16) so k/v projections need no PE transposes
and stream at 1 cycle/row; q is host-transposed (f16) and preloaded whole.
The projected k/v PSUM tile is immediately copied to SBUF as bf16 (split
ACT/DVE) so every subsequent elementwise op runs from SBUF in 16-bit, where the
DVE gets its 2-4x fast modes and GPSIMD can help (GPSIMD cannot touch PSUM).
Two phases (attention+Wo+LN-stats, then LN-apply+MLP) keep the scalar engine on
one activation table per phase. The MLP tail is batched 4 tiles per group with
2-tile PE transposes whose [64,128] results are copied as two base-0 SBUF
tiles (matmuls from base partition 32/64 operands fault on HW). Output
accumulates in SBUF, ships as one DMA, host unshuffles.
"""

import math

import numpy as np

B, DQ, DKV, T, A, H, O = 131072, 13, 32, 24, 32, 4, 32
HD = A // H
LN_EPS = 1e-5
NCORES = 8
BP = B // NCORES  # rows per core
P = 128
NT = BP // P      # tiles per core
NC = 6            # kv feature chunks (24 tokens * 32 dkv / 128)
G4 = 4            # tiles per batch-group
USE_GPSIMD = True  # offload some SBUF-only elementwise ops to the Pool engine
CSPLIT = 1        # kv chunks copied PSUM->SBUF by DVE (rest by ACT)

_CACHE = {}


def _ap(base, dims, extra_offset=0):
    """Build an AP over base (an AP) with given free [step, count] dims."""
    import concourse.bass as bass
    return bass.AP(tensor=base.tensor, offset=base.offset + extra_offset,
                   ap=[base.ap[0]] + [list(d) for d in dims])


def _build(nt=NT, use_gpsimd=None):
    import concourse.bacc as bacc
    import concourse.tile as tile
    from concourse import mybir

    if use_gpsimd is None:
        use_gpsimd = USE_GPSIMD

    f32 = mybir.dt.float32
    f16 = mybir.dt.float16
    AF = mybir.ActivationFunctionType
    OP = mybir.AluOpType
    AX = mybir.AxisListType

    nc = bacc.Bacc()
    ew = nc.gpsimd if use_gpsimd else nc.vector

    qt_d = nc.dram_tensor("qt", [16, BP], f16, kind="ExternalInput")
    kv_d = nc.dram_tensor("kv_in", [nt, P, NC * P], f16, kind="ExternalInput")
    ident_d = nc.dram_tensor("ident", [128, 128], f16, kind="ExternalInput")
    wq_d = nc.dram_tensor("wq", [16, 32], f16, kind="ExternalInput")
    wkv_d = nc.dram_tensor("wkv", [128, 256], f16, kind="ExternalInput")
    wo_d = nc.dram_tensor("wo", [128, 32], f16, kind="ExternalInput")
    wd1_d = nc.dram_tensor("wd1", [128, 32], f16, kind="ExternalInput")
    wd2_d = nc.dram_tensor("wd2", [128, 32], f16, kind="ExternalInput")
    lnw_d = nc.dram_tensor("lnw", [128, 32], f32, kind="ExternalInput")
    b1_d = nc.dram_tensor("b1", [128, 32], f32, kind="ExternalInput")
    lnb_d = nc.dram_tensor("lnb", [128, 32], f32, kind="ExternalInput")
    out_d = nc.dram_tensor("out", [P, nt * O], f32, kind="ExternalOutput")

    ng = nt // G4

    with tile.TileContext(nc) as tc:
        with (
            tc.tile_pool(name="consts", bufs=1) as consts,
            tc.tile_pool(name="kvload", bufs=5) as kvload,
            tc.tile_pool(name="kvsb", bufs=4) as kvsb,
            tc.tile_pool(name="bigs", bufs=4) as bigs,
            tc.tile_pool(name="trsb", bufs=4) as trsb,
            tc.tile_pool(name="smalls", bufs=6) as smalls,
            tc.tile_pool(name="kvpp", bufs=2, space="PSUM") as kvpp,
            tc.tile_pool(name="sp", bufs=2, space="PSUM") as sp,
        ):
            ident_sb = consts.tile([128, 128], f16)
            wq_sb = consts.tile([16, 32], f16)
            wkv_sb = consts.tile([128, 256], f16)
            wo_sb = consts.tile([128, 32], f16)
            wd1_sb = consts.tile([128, 32], f16)
            wd2_sb = consts.tile([128, 32], f16)
            lnw_sb = consts.tile([128, 32], f32)
            b1_sb = consts.tile([128, 32], f32)
            lnb_sb = consts.tile([128, 32], f32)
            qt_all = consts.tile([16, BP], f16)

            qproj_all = consts.tile([128, nt, 32], f16)
            out1_all = consts.tile([128, nt, 32], f32)
            mean_all = consts.tile([128, nt], f32)
            var_all = consts.tile([128, nt], f32)
            std_all = consts.tile([128, nt], f32)
            rstd_all = consts.tile([128, nt], f32)
            out_all = consts.tile([128, nt, 32], f32)
            z_all = consts.tile([128, nt, 32], f16)
            h1pre_all = consts.tile([128, nt, 32], f32)
            eps_sb = consts.tile([128, 1], f32)
            nc.vector.memset(eps_sb, LN_EPS)

            nc.sync.dma_start(out=ident_sb, in_=ident_d[:, :])
            nc.sync.dma_start(out=wq_sb, in_=wq_d[:, :])
            nc.sync.dma_start(out=wkv_sb, in_=wkv_d[:, :])
            nc.sync.dma_start(out=wo_sb, in_=wo_d[:, :])
            nc.sync.dma_start(out=wd1_sb, in_=wd1_d[:, :])
            nc.sync.dma_start(out=wd2_sb, in_=wd2_d[:, :])
            nc.sync.dma_start(out=lnw_sb, in_=lnw_d[:, :])
            nc.sync.dma_start(out=b1_sb, in_=b1_d[:, :])
            nc.sync.dma_start(out=lnb_sb, in_=lnb_d[:, :])
            nc.sync.dma_start(out=qt_all, in_=qt_d[:, :])

            # ---- phase A0: q projections, batched bf16 copies ----
            for g in range(ng):
                q4_ps = sp.tile([128, G4, 32], f32, tag="sp")
                for jj in range(G4):
                    i = g * G4 + jj
                    nc.tensor.matmul(q4_ps[:, jj, :],
                                     lhsT=qt_all[:, i * P:(i + 1) * P],
                                     rhs=wq_sb)
                nc.scalar.copy(qproj_all[:, g * G4:(g + 1) * G4, :], q4_ps)

            # ---- phase A: attention + Wo + LN stats ----
            for g in range(ng):
                ctx4 = trsb.tile([128, G4, 32], f16, tag="ctx4")
                for jj in range(G4):
                    i = g * G4 + jj
                    kv_t = kvload.tile([P, NC * P], f16, tag="kv")
                    nc.sync.dma_start(out=kv_t, in_=kv_d[i])

                    kvp = kvpp.tile([P, NC, 4, 2, 32], f32, tag="kvp")
                    for c in range(NC):
                        nc.tensor.matmul(kvp[:, c, :, :, :],
                                         lhsT=kv_t[:, c * P:(c + 1) * P],
                                         rhs=wkv_sb)

                    # ACT copies k,v out of PSUM into fp16 SBUF so the DVE
                    # muls run in 2x packed mode. v is reordered to [H,HD,T]
                    # so the ctx reduce is contiguous-innermost.
                    k_ap = _ap(kvp, [[8, H], [64, T], [1, HD]])
                    v_ap = _ap(kvp, [[8, H], [1, HD], [64, T]],
                               extra_offset=32)
                    ksb = kvsb.tile([P, H, T, HD], f16, tag="ksb")
                    nc.scalar.copy(ksb, k_ap)
                    vsb = kvsb.tile([P, H, HD, T], f16, tag="vsb")
                    nc.scalar.copy(vsb, v_ap)

                    # scores = sum_d q*k, layout [h, t]
                    prod = bigs.tile([P, H, T, HD], f16, tag="prod")
                    qb_ap = _ap(qproj_all, [[8, H], [0, T], [1, HD]],
                                extra_offset=i * 32)
                    nc.vector.tensor_mul(prod, ksb, qb_ap)
                    scores = smalls.tile([P, H, T], f16, tag="scores")
                    with nc.allow_low_precision(reason="8-term f16 sums"):
                        nc.vector.reduce_sum(scores, prod, axis=AX.X)

                    # softmax over t (no max-sub; |scores| <~ 5)
                    exps = smalls.tile([P, H, T], f16, tag="exps")
                    nc.scalar.activation(exps, scores, AF.Exp)
                    denom = smalls.tile([P, H], f32, tag="denom")
                    nc.vector.reduce_sum(denom, exps, axis=AX.X)

                    # ctx = sum_t attn*v; prod2 [H, HD, T] (t contiguous),
                    # split DVE (h 0-1) / GPSIMD (h 2-3)
                    prod2 = bigs.tile([P, H, HD, T], f16, tag="prod2")
                    e_lo = _ap(exps, [[T, 2], [0, HD], [1, T]])
                    e_hi = _ap(exps, [[T, 2], [0, HD], [1, T]],
                               extra_offset=2 * T)
                    nc.vector.tensor_mul(prod2[:, 0:2], vsb[:, 0:2], e_lo)
                    nc.gpsimd.tensor_mul(prod2[:, 2:4], vsb[:, 2:4], e_hi)
                    ctxu = smalls.tile([P, A], f32, tag="ctxu")
                    nc.vector.reduce_sum(ctxu, prod2, axis=AX.X)
                    # ctx = ctxu * (1/denom)  (broadcast over head_dim)
                    rden = smalls.tile([P, H], f32, tag="rden")
                    nc.vector.reciprocal(rden, denom)
                    ew.tensor_tensor(
                        out=ctx4[:, jj, :], in0=ctxu,
                        in1=_ap(rden, [[1, H], [0, HD]]), op=OP.mult)

                # 2-tile transposes; copies split the [64,128] PSUM result
                # into two base-0 SBUF tiles (offset matmuls fault on HW)
                out1_ps = sp.tile([128, G4, O], f32, tag="sp")
                for h2 in range(2):
                    ctxT_ps = sp.tile([64, 128], f16, tag="sp")
                    nc.tensor.transpose(ctxT_ps, ctx4[:, 2 * h2:2 * h2 + 2, :],
                                        ident_sb)
                    for sj in range(2):
                        jj = 2 * h2 + sj
                        ctxT_sb = trsb.tile([32, 128], f16, tag="ctxT")
                        nc.scalar.copy(ctxT_sb, ctxT_ps[sj * 32:sj * 32 + 32])
                        nc.tensor.matmul(out1_ps[:, jj, :], lhsT=ctxT_sb,
                                         rhs=wo_sb[0:32, :])
                o4 = out1_all[:, g * G4:(g + 1) * G4, :]
                nc.vector.tensor_copy(o4, out1_ps)

                # batched LN stats: mean, var = E[x^2] - mean^2
                sums = smalls.tile([P, G4], f32, tag="sums")
                nc.vector.reduce_sum(sums, o4, axis=AX.X)
                sq4 = trsb.tile([128, G4, O], f16, tag="sq4")
                ew.tensor_mul(sq4, o4, o4)
                sqs = smalls.tile([P, G4], f32, tag="sqs")
                nc.vector.reduce_sum(sqs, sq4, axis=AX.X)
                mean4 = mean_all[:, g * G4:(g + 1) * G4]
                ew.tensor_scalar(mean4, sums, 1.0 / O, None,
                                        op0=OP.mult)
                msq = smalls.tile([P, G4], f32, tag="msq")
                ew.tensor_mul(msq, mean4, mean4)
                ew.tensor_scalar(var_all[:, g * G4:(g + 1) * G4], sqs,
                                        1.0 / O, None, op0=OP.mult)
                ew.tensor_sub(var_all[:, g * G4:(g + 1) * G4],
                                     var_all[:, g * G4:(g + 1) * G4], msq)

                # ---- B-pre (rstd deferred): z = (out1-mean)*lnw, then
                # z @ Wd1 now; rstd and lnb@Wd1 are applied at gelu time
                # because rstd is a per-row scalar that commutes with Wd1.
                sl = slice(g * G4, (g + 1) * G4)
                mean_b = _ap(mean_all, [[1, G4], [0, O]], extra_offset=g * G4)
                lnw_b = _ap(lnw_sb, [[0, G4], [1, O]])
                xc4 = smalls.tile([P, G4, O], f32, tag="xc4")
                nc.vector.tensor_tensor(out=xc4, in0=out1_all[:, sl, :],
                                        in1=mean_b, op=OP.subtract)
                ew.tensor_tensor(out=z_all[:, sl, :], in0=xc4, in1=lnw_b,
                                 op=OP.mult)
                h1_ps = sp.tile([128, G4, O], f32, tag="sp")
                for h2 in range(2):
                    zT_ps = sp.tile([64, 128], f16, tag="sp")
                    nc.tensor.transpose(
                        zT_ps, z_all[:, g * G4 + 2 * h2:g * G4 + 2 * h2 + 2, :],
                        ident_sb)
                    for sj in range(2):
                        jj = 2 * h2 + sj
                        zT_sb = trsb.tile([32, 128], f16, tag="lnT")
                        nc.scalar.copy(zT_sb, zT_ps[sj * 32:sj * 32 + 32])
                        nc.tensor.matmul(h1_ps[:, jj, :], lhsT=zT_sb,
                                         rhs=wd1_sb[0:32, :])
                nc.scalar.copy(h1pre_all[:, sl, :], h1_ps)

            # ---- batched rstd (one Sqrt table load) ----
            nc.scalar.activation(std_all, var_all, AF.Sqrt, bias=eps_sb)
            nc.vector.reciprocal(rstd_all, std_all)

            # ---- tail: apply rstd + bias, gelu, Wd2, gelu, residual ----
            for g in range(ng):
                sl = slice(g * G4, (g + 1) * G4)
                rstd_b = _ap(rstd_all, [[1, G4], [0, O]], extra_offset=g * G4)
                lnb_b = _ap(lnb_sb, [[0, G4], [1, O]])
                b1_b = _ap(b1_sb, [[0, G4], [1, O]])

                gin4 = smalls.tile([P, G4, O], f32, tag="gin4")
                nc.vector.tensor_tensor(out=gin4, in0=h1pre_all[:, sl, :],
                                        in1=rstd_b, op=OP.mult)
                nc.vector.tensor_tensor(out=gin4, in0=gin4, in1=b1_b,
                                        op=OP.add)
                h1_4 = trsb.tile([128, G4, O], f16, tag="h1_4")
                nc.scalar.activation(h1_4, gin4, AF.Gelu)

                h2_ps = sp.tile([128, G4, O], f32, tag="sp")
                for h2 in range(2):
                    h1T_ps = sp.tile([64, 128], f16, tag="sp")
                    nc.tensor.transpose(h1T_ps, h1_4[:, 2 * h2:2 * h2 + 2, :],
                                        ident_sb)
                    for sj in range(2):
                        jj = 2 * h2 + sj
                        h1T_sb = trsb.tile([32, 128], f16, tag="h1T")
                        nc.scalar.copy(h1T_sb, h1T_ps[sj * 32:sj * 32 + 32])
                        nc.tensor.matmul(h2_ps[:, jj, :], lhsT=h1T_sb,
                                         rhs=wd2_sb[0:32, :])
                h2_4 = smalls.tile([128, G4, O], f16, tag="h2_4")
                nc.scalar.activation(h2_4, h2_ps, AF.Gelu)

                # out = z*rstd + (lnb + h2)
                zr4 = smalls.tile([P, G4, O], f32, tag="zr4")
                nc.vector.tensor_tensor(out=zr4, in0=z_all[:, sl, :],
                                        in1=rstd_b, op=OP.mult)
                h2l4 = smalls.tile([P, G4, O], f32, tag="h2l4")
                ew.tensor_tensor(out=h2l4, in0=h2_4, in1=lnb_b, op=OP.add)
                nc.vector.tensor_add(out_all[:, sl, :], zr4, h2l4)

            nc.sync.dma_start(out=out_d[:, :], in_=out_all)

    nc.compile()
    return nc


def _prep_weights(Wq, Wk, Wv, Wo, ln_w, ln_b, Wd1, Wd2):
    bf = np.float16
    s = 1.0 / math.sqrt(HD)
    wq = np.zeros((16, 32), np.float32)
    wq[:DQ] = np.asarray(Wq, np.float32) * s
    wkv = np.zeros((128, 256), np.float32)
    Wk = np.asarray(Wk, np.float32)
    Wv = np.asarray(Wv, np.float32)
    for tl in range(4):
        wkv[tl * 32:(tl + 1) * 32, tl * 64:tl * 64 + 32] = Wk
        wkv[tl * 32:(tl + 1) * 32, tl * 64 + 32:tl * 64 + 64] = Wv
    lnw = np.broadcast_to(np.asarray(ln_w, np.float32), (128, 32)).copy()
    lnb = np.broadcast_to(np.asarray(ln_b, np.float32), (128, 32)).copy()
    b1v = np.asarray(ln_b, np.float32) @ np.asarray(Wd1, np.float32)
    b1 = np.broadcast_to(b1v, (128, 32)).copy()
    return {
        "ident": np.eye(128, dtype=bf),
        "wq": wq.astype(bf),
        "wkv": wkv.astype(bf),
        "wo": np.tile(np.asarray(Wo, np.float32), (4, 1)).astype(bf),
        "wd1": np.tile(np.asarray(Wd1, np.float32), (4, 1)).astype(bf),
        "wd2": np.tile(np.asarray(Wd2, np.float32), (4, 1)).astype(bf),
        "lnw": lnw,
        "lnb": lnb,
        "b1": b1,
    }


def kernel(query, kv, Wq, Wk, Wv, Wo, ln_w, ln_b, Wd1, Wd2):
    from concourse.bass_utils import run_bass_kernel_spmd

    bf = np.float16

    if "nc" not in _CACHE:
        _CACHE["nc"] = _build()
    nc = _CACHE["nc"]

    query = np.asarray(query, np.float32)
    kv = np.asarray(kv, np.float32)
    w = _prep_weights(Wq, Wk, Wv, Wo, ln_w, ln_b, Wd1, Wd2)

    # q: [B, DQ] -> per-core [16, BP] transposed + padded, bf16
    qt = np.zeros((NCORES, 16, BP), bf)
    qt[:, :DQ, :] = query.reshape(NCORES, BP, DQ).transpose(0, 2, 1).astype(bf)
    # kv: [B, T, DKV] -> per-core [NT, 128(feat r*32+d), 6(c), 128(b)], bf16
    kvp = np.ascontiguousarray(
        kv.reshape(NCORES, NT, P, NC, 4, DKV).transpose(0, 1, 4, 5, 3, 2)
        .astype(bf)
    ).reshape(NCORES, NT, P, NC * P)

    in_maps = []
    for c in range(NCORES):
        m = dict(w)
        m["qt"] = qt[c]
        m["kv_in"] = kvp[c]
        in_maps.append(m)

    res = run_bass_kernel_spmd(nc, in_maps, core_ids=list(range(NCORES)),
                               trace=False)
    _CACHE["last_results"] = res
    out = np.concatenate(
        [r["out"].reshape(P, NT, O).transpose(1, 0, 2).reshape(BP, O)
         for r in res.results], axis=0)
    return out
